# revision 30
# baseline (speedup 1.0000x reference)
"""GPT-2 forward on 8 TRN2 NeuronCores — strided context-parallel Bass/Tile kernel.

Sharding: 4 sequences x 2 cores each. Core 2b+p owns tokens of sequence b at
global positions {2u+p : u in [0, S/2)} (strided interleave), so the causal
block structure is identical on every core. Per layer, each pair AllGathers
its (k^T, v) shard; gathered halves are loaded with one DMA per (rank,
tensor) into chunk-blocked layouts (key chunk j = [64 rank-0 keys | 64
rank-1 keys], key global position = 128j + 2s + r). k travels the wire in
fp8e4 (upconverted to bf16 in the restripe copy); v stays bf16.

Pipeline shape (v2):
  - LN statistics are computed INSIDE the producing matmul loop (proj for
    ln2, fc2 for the next layer's ln1) so the stats chain overlaps PE work;
    only the (x-m)*rstd apply + DMA-transposes remain at phase start.
    The little stat ops (m, s1*m, s2-t2) run on the idle GPSIMD engine so
    they never block the DVE add queue.
  - qkv/bias weight loads are issued before the LN apply, so they prefetch
    during the previous layer's fc2.
  - Attention: scores per (head-pair, key-chunk) with row-split (even head
    partitions 0-63, odd 64-127), exp on ACT over [128, 2, N] psum
    (query-truncated at qlo=64j), causal diagonal handled by a post-exp
    0/1-mask multiply on DVE (no PE mask matmuls), av with a ones-column
    appended to v so softmax denominators fall out of the same matmul.
    Normalization r = exp(-ln(denom)) broadcast via two rank-1 [64,T] PE
    matmuls allocated from the (attention-idle) psA ring, keeping the
    scores psum ring purely double-buffered.
  - att tiles get a 6-deep pool so scores/exp of later chunks can run far
    ahead of AV while the v AllGather is still in flight.

LN gamma/beta and the 1/sqrt(D) attention scale are folded into weights on
the host; biases that are identically zero are skipped at build time.
"""
import sys, os, functools
sys.path.insert(0, '/opt/trn_rl_repo')
import numpy as np
import ml_dtypes
import concourse.bass as bass
import concourse.mybir as mybir
from concourse import bacc
from concourse import hw_specs
from concourse.bass_utils import run_bass_kernel_spmd
from concourse.tile import TileContext

F32 = mybir.dt.float32
BF16 = mybir.dt.bfloat16
FP8 = mybir.dt.float8e4
AF = mybir.ActivationFunctionType
ALU = mybir.AluOpType
BF = ml_dtypes.bfloat16
E8 = ml_dtypes.float8_e4m3

# The act-table placement pass picks the first set containing each function,
# so Ln -> natural_log and Exp -> exp_and_others, forcing a ~1.5us table swap
# per transition. Strip ln/exp from every set except the combined one (set
# order/IDs unchanged) so both resolve to natural_log_exp_and_others.
_orig_gat = hw_specs.get_activation_tables


@functools.cache
def _patched_gat(arch):
    t = _orig_gat(arch)
    if 'natural_log_exp_and_others' not in t:
        return t
    out = {}
    for name, fns in t.items():
        fns = set(fns)
        if name != 'natural_log_exp_and_others':
            fns.discard(AF.Exp)
            fns.discard(AF.Ln)
        out[name] = fns
    return out


hw_specs.get_activation_tables = _patched_gat
bacc.get_activation_tables = _patched_gat


def cfg_full():
    return dict(B=4, S=1024, L=12, H=12, D=64, F=3072, V=50257)


def cfg_mini():
    return dict(B=4, S=256, L=2, H=2, D=64, F=256, V=640)


def derived(c):
    d = dict(c)
    d['E'] = c['H'] * c['D']
    d['T'] = c['S'] // 2          # local tokens per core
    d['QCH'] = d['T'] // 128      # query (token) chunks
    d['KCH'] = c['S'] // 128      # global key chunks
    d['ECH'] = d['E'] // 128      # embed chunks
    d['FCH'] = c['F'] // 128      # mlp hidden chunks
    d['VNC'] = (c['V'] + 511) // 512  # lm-head n-chunks
    assert d['T'] % 128 == 0 and d['E'] % 128 == 0 and c['F'] % 128 == 0
    return d


def build(c, has_bias):
    d = derived(c)
    T, E, H, D, F, V, L = d['T'], d['E'], c['H'], c['D'], c['F'], c['V'], c['L']
    QCH, KCH, ECH, FCH, VNC = d['QCH'], d['KCH'], d['ECH'], d['FCH'], d['VNC']
    NPAIR = H // 2

    nc = bacc.Bacc("TRN2", target_bir_lowering=False, debug=False, num_devices=8)

    # ---- dram parameters ----
    h0_p = nc.declare_dram_parameter("h0", [T, E], F32, isOutput=False)
    wqkv_p = nc.declare_dram_parameter("wqkv", [L, 128, ECH, 3 * E], BF16, isOutput=False)
    wproj_p = nc.declare_dram_parameter("wproj", [L, 128, ECH, E], BF16, isOutput=False)
    wfc_p = nc.declare_dram_parameter("wfc", [L, 128, ECH, F], BF16, isOutput=False)
    wfc2_p = nc.declare_dram_parameter("wfc2", [L, 128, FCH, E], BF16, isOutput=False)
    wlm_p = nc.declare_dram_parameter("wlm", [128, ECH, VNC * 512], BF16, isOutput=False)
    mask01_p = nc.declare_dram_parameter("mask01", [128, 2, 64], BF16, isOutput=False)
    i128_p = nc.declare_dram_parameter("i128", [128, 128], BF16, isOutput=False)
    if has_bias['qkv']:
        bqk_p = nc.declare_dram_parameter("bqk", [L, 2 * ECH, 128, 1], F32, isOutput=False)
    if has_bias['v']:
        bv_p = nc.declare_dram_parameter("bv", [L, 1, E], BF16, isOutput=False)
    if has_bias['proj']:
        bproj_p = nc.declare_dram_parameter("bproj", [L, 128, E], F32, isOutput=False)
    if has_bias['fc']:
        bfc_p = nc.declare_dram_parameter("bfc", [L, FCH, 128, 1], F32, isOutput=False)
    if has_bias['fc2']:
        bfc2_p = nc.declare_dram_parameter("bfc2", [L, 128, E], F32, isOutput=False)
    if has_bias['lm']:
        blm_p = nc.declare_dram_parameter("blm", [1, VNC * 512], BF16, isOutput=False)
    out_p = nc.declare_dram_parameter("logits", [T, V], BF16, isOutput=True)

    with TileContext(nc) as tc:
        with (
            tc.tile_pool(name="persist", bufs=1) as persist,
            tc.tile_pool(name="acts", bufs=1) as acts,
            tc.tile_pool(name="wpool", bufs=2) as wpool,
            tc.tile_pool(name="stage", bufs=3) as stage,
            tc.tile_pool(name="attp", bufs=6) as attp,
            tc.tile_pool(name="norm", bufs=2) as norm,
            tc.tile_pool(name="small", bufs=5) as small,
            tc.tile_pool(name="psA", bufs=2, space="PSUM") as psA,
            tc.tile_pool(name="psSc", bufs=2, space="PSUM") as psSc,
            tc.tile_pool(name="psY", bufs=2, space="PSUM") as psY,
            tc.tile_pool(name="dramcc", bufs=2, space="DRAM") as dcc,
        ):
            # ---- persistent tiles ----
            h_sb = persist.tile([128, QCH, E], F32, tag="h")
            nc.sync.dma_start(h_sb[:], h0_p.ap().rearrange("(q p) e -> p q e", p=128))
            mask01_sb = persist.tile([128, 2, 64], BF16, tag="mask01")
            nc.sync.dma_start(mask01_sb[:], mask01_p.ap())
            i128_sb = persist.tile([128, 128], BF16, tag="i128")
            nc.sync.dma_start(i128_sb[:], i128_p.ap())
            ones_sb = persist.tile([128, 128], BF16, tag="ones")
            nc.gpsimd.memset(ones_sb[:], 1.0)
            # gathered k/v, persistent buffers. kT travels the wire in fp8e4
            # through a contiguous staging tile; the DVE restripe upcasts to
            # bf16. v's ones column travels through the collective (cc buffers
            # initialized once from v_aug's ones).
            kT_all = persist.tile([128, ECH, KCH, 2, 64], BF16, tag="kTall")
            v_aug = persist.tile([128, KCH, H, 65], BF16, tag="vaug")
            nc.gpsimd.memset(v_aug[:], 1.0)
            kstage_out = persist.tile([128, ECH, T], FP8, tag="kstout")
            eps_sb = persist.tile([128, 1], F32, tag="eps")
            nc.gpsimd.memset(eps_sb[:], 1e-5)

            def ln_stats(x):
                """Emit LN stat ops for one [128, E] f32 tile; returns (m, rstd).

                reduce on DVE, Square-accum on ACT, the tiny combine ops on
                GPSIMD (so the DVE queue never blocks on ACT), Ln/Exp on ACT.
                rstd = exp(-0.5 ln(var+eps)) keeps everything in the
                natural_log_exp ACT table set.
                """
                s1 = small.tile([128, 1], F32, tag="ln_s1")
                nc.vector.tensor_reduce(s1[:], x, mybir.AxisListType.X, ALU.add)
                s2 = small.tile([128, 1], F32, tag="ln_s2")
                trash = acts.tile([128, E], BF16, tag="ln_trash")
                nc.scalar.activation(trash[:], x, AF.Square, accum_out=s2[:])
                m = small.tile([128, 1], F32, tag="ln_m")
                nc.gpsimd.tensor_scalar_mul(m[:], s1[:], 1.0 / E)
                t2 = small.tile([128, 1], F32, tag="ln_t2")
                nc.gpsimd.tensor_tensor(t2[:], s1[:], m[:], ALU.mult)
                t3 = small.tile([128, 1], F32, tag="ln_t3")
                nc.gpsimd.tensor_tensor(t3[:], s2[:], t2[:], ALU.subtract)
                lnv = small.tile([128, 1], F32, tag="ln_lnv")
                nc.scalar.activation(lnv[:], t3[:], AF.Ln, bias=eps_sb[:], scale=1.0 / E)
                rstd = small.tile([128, 1], F32, tag="ln_rstd")
                nc.scalar.activation(rstd[:], lnv[:], AF.Exp, scale=-0.5)
                return m, rstd

            def ln_apply(stats, out_tag):
                """(x-m)*rstd per tile -> bf16, transposed to [128, ECH, T]
                via PE identity-matmul transposes (+ DVE psum->sbuf copies).
                Keeps the transpose off the DMA queues (which may be busy
                streaming weights) and feeds the PE during the LN boundary;
                emitted c-major so aT[:, 0, :] completes first and the next
                matmul phase can start early."""
                lnout = acts.tile([128, QCH, E], BF16, tag="lnout")
                tp = acts.tile([128, ECH, T], BF16, tag=out_tag)
                for t in range(QCH):
                    m, rstd = stats[t]
                    nc.vector.tensor_scalar(
                        lnout[:, t, :], h_sb[:, t, :], m[:], rstd[:],
                        ALU.subtract, ALU.mult)
                for c in range(ECH):
                    for t in range(QCH):
                        tps = psSc.tile([128, 128], BF16, tag="sc")
                        nc.tensor.transpose(
                            tps[:], lnout[:, t, 128 * c:128 * (c + 1)], i128_sb[:])
                        nc.vector.tensor_copy(
                            out=tp[:, c, 128 * t:128 * (t + 1)], in_=tps[:])
                return tp

            # cc exchange buffers: single persistent DRAM tiles reused every
            # layer (layers are serial). The softmax-ones columns of the v
            # buffer are seeded ONCE here instead of re-shipping 800KB of
            # seed every layer (which congested the k-AllGather's DMA window).
            VSZ = 64 * KCH * H * 65
            cck_in = dcc.tile([E * T], FP8, tag="cck_in", bufs=1)
            cck_out = dcc.tile([2, E * T], FP8, tag="cck_out", bufs=1)
            ccv_in = dcc.tile([VSZ], BF16, tag="ccv_in", bufs=1)
            ccv_out = dcc.tile([2, VSZ], BF16, tag="ccv_out", bufs=1)
            cc_v = ccv_in[:].rearrange("(s j h d) -> s j h d", s=64, j=KCH, h=H)
            nc.sync.dma_start(cc_v[:, :, :, 64:65], v_aug[0:64, :, :, 64:65])

            # initial ln1 stats (layer 0) straight off the embedded input
            stats = [ln_stats(h_sb[:, t, :]) for t in range(QCH)]

            for l in range(L):
                # -------- weight prefetch (overlaps previous fc2) --------
                ECH_H = max(ECH // 2, 1)
                wq_t = []
                for hw in range(ECH // ECH_H):
                    wt = wpool.tile([128, ECH_H, 3 * E], BF16, tag="W")
                    nc.scalar.dma_start(wt[:], wqkv_p[l, :, hw * ECH_H:(hw + 1) * ECH_H, :])
                    wq_t.append(wt)
                wq_at = lambda kc: (wq_t[kc // ECH_H], kc % ECH_H)
                if has_bias['qkv']:
                    bqk_sb = small.tile([128, 2 * ECH], F32, tag="bqk")
                    nc.sync.dma_start(bqk_sb[:], bqk_p[l].rearrange("c p one -> p (c one)"))
                if has_bias['v']:
                    bv_sb = small.tile([1, E], BF16, tag="bv")
                    nc.sync.dma_start(bv_sb[:], bv_p[l])

                # -------- ln1 apply --------
                aT = ln_apply(stats, "xT")

                # -------- qkv --------
                qT = acts.tile([128, ECH, T], BF16, tag="qT")

                def qk_chunk(mc):
                    # mc in [0, ECH): q chunk -> qT ; [ECH, 2*ECH): k chunk -> cc_in
                    ps = psA.tile([128, T], F32, tag="mm")
                    for kc in range(ECH):
                        wt, kk = wq_at(kc)
                        nc.tensor.matmul(ps[:], wt[:, kk, 128 * mc:128 * (mc + 1)],
                                         aT[:, kc, :], start=(kc == 0), stop=(kc == ECH - 1))
                    if mc < ECH:
                        dst = qT[:, mc, :]
                    else:
                        dst = kstage_out[:, mc - ECH, :]
                    if has_bias['qkv']:
                        nc.vector.tensor_scalar_add(dst, ps[:], bqk_sb[:, mc:mc + 1])
                    else:
                        nc.vector.tensor_copy(out=dst, in_=ps[:])
                for mc in range(ECH, 2 * ECH):  # k chunks: computed and shipped first
                    qk_chunk(mc)
                nc.sync.dma_start(
                    cck_in[:].rearrange("(p a) -> p a", p=128),
                    kstage_out[:].rearrange("p q t -> p (q t)"))
                # k exchange fires as soon as k chunks land; overlaps v compute
                nc.gpsimd.collective_compute(
                    "AllGather", ALU.bypass,
                    replica_groups=[[0, 1], [2, 3], [4, 5], [6, 7]],
                    ins=[cck_in[:]], outs=[cck_out[:]])
                # v chunks: computed now, but the descriptor-heavy strided
                # stores to the cc buffer are DEFERRED until after the k
                # AllGather (see below) so they can't congest its DMA window.
                # bufs=8 keeps all (t, nn) staging tiles live meanwhile.
                H2 = H // 2
                vstgs = []
                for t in range(QCH):
                    for nn in range(2):
                        NW = E // 2
                        ps = psA.tile([128, NW], F32, tag="mm")
                        for kc in range(ECH):
                            wt, kk = wq_at(kc)
                            nc.tensor.matmul(ps[:], aT[:, kc, 128 * t:128 * (t + 1)],
                                             wt[:, kk, 2 * E + nn * NW: 2 * E + (nn + 1) * NW],
                                             start=(kc == 0), stop=(kc == ECH - 1 and not has_bias['v']))
                        if has_bias['v']:
                            nc.tensor.matmul(ps[:], ones_sb[0:1, 0:128],
                                             bv_sb[0:1, nn * NW:(nn + 1) * NW],
                                             start=False, stop=True)
                        vstg = stage.tile([128, NW], BF16, tag="vstg", bufs=8)
                        nc.vector.tensor_copy(out=vstg[:], in_=ps[:])
                        vstgs.append((t, nn, vstg))

                for mc in range(ECH):  # q chunks: no dep on the collectives
                    qk_chunk(mc)

                # gathered k loads: contiguous DMAs; kT rank-interleave +
                # fp8 -> bf16 upconvert on DVE
                for r in range(2):
                    kstage = norm.tile([128, ECH, KCH, 64], FP8, tag="kstage")
                    nc.sync.dma_start(
                        kstage[:],
                        cck_out[r].rearrange("(p q j s) -> p q j s", p=128, q=ECH, j=KCH))
                    nc.vector.tensor_copy(out=kT_all[:, :, :, r, :], in_=kstage[:])
                # v stores: behind the kstage loads on the sync FIFO, so the
                # ~6k small descriptors only hit the DMA queues after the k-AG
                # has drained (they'd otherwise triple its latency). The
                # queues are idle during the scores phase that follows.
                for t, nn, vstg in vstgs:
                    for jp in range(2):
                        nc.sync.dma_start(
                            cc_v[:, 2 * t + jp, nn * H2:(nn + 1) * H2, 0:64],
                            vstg[64 * jp:64 * jp + 64, :]
                            .rearrange("s (h d) -> s h d", h=H2))
                # ---------------- v exchange (pairs) ----------------
                nc.gpsimd.collective_compute(
                    "AllGather", ALU.bypass,
                    replica_groups=[[0, 1], [2, 3], [4, 5], [6, 7]],
                    ins=[ccv_in[:]], outs=[ccv_out[:]])
                for r in range(2):
                    nc.sync.dma_start(
                        v_aug[64 * r:64 * (r + 1), :, :, :],
                        ccv_out[r].rearrange("(s j h dd) -> s j h dd", s=64, j=KCH, h=H))
                # proj + fc1 weight prefetch: issued on the SYNC queue behind
                # the v_aug loads so the multi-MB streams only hit the DMA
                # queues after both collectives have drained (they'd otherwise
                # stall the CC's chunk DMAs and the attention start).
                wp = wpool.tile([128, ECH, E], BF16, tag="W")
                nc.sync.dma_start(wp[:], wproj_p[l])
                wf = wpool.tile([128, ECH, F], BF16, tag="W")
                nc.sync.dma_start(wf[:], wfc_p[l])

                # ---------------- attention ----------------
                yT_c = acts.tile([128, ECH, T], BF16, tag="yTc")
                for cpair in range(NPAIR):
                    yps_e = psY.tile([65, T], F32, tag="yps")
                    yps_o = psY.tile([65, T], F32, tag="yps")
                    for j in range(KCH):
                        qlo = min(64 * j, T - 64)
                        sc = psSc.tile([128, 2, T], F32, tag="sc")
                        att = attp.tile([128, 2, T], BF16, tag="att")
                        for hh in range(2):
                            plo = 64 * hh
                            nc.tensor.matmul(
                                sc[:, hh, qlo:T],
                                kT_all[plo:plo + 64, cpair, j, :, :].rearrange("p a b -> p (a b)"),
                                qT[plo:plo + 64, cpair, qlo:T],
                                start=True, stop=True)
                        nc.scalar.activation(att[:, :, qlo:T], sc[:, :, qlo:T], AF.Exp)
                        # causal diagonal: post-exp 0/1 multiply on DVE
                        nc.vector.tensor_tensor(
                            att[:, :, qlo:qlo + 64], att[:, :, qlo:qlo + 64],
                            mask01_sb[:], ALU.mult)
                        for hh, yps in ((0, yps_e), (1, yps_o)):
                            nc.tensor.matmul(yps[:, qlo:T], v_aug[:, j, 2 * cpair + hh, :],
                                             att[:, hh, qlo:T],
                                             start=(j == 0), stop=(j == KCH - 1))
                    # softmax denominators -> r = exp(-ln(d)); the rank-1
                    # broadcasts go to [64, T] tiles in the psA ring (idle
                    # during attention) so the scores ring stays double-buffered
                    dstage = norm.tile([65, 2, T], BF16, tag="dstage")
                    nc.vector.tensor_copy(out=dstage[64:65, 0, :], in_=yps_e[64:65, :])
                    nc.vector.tensor_copy(out=dstage[64:65, 1, :], in_=yps_o[64:65, :])
                    rps_e = psA.tile([64, T], F32, tag="mm")
                    rps_o = psA.tile([64, T], F32, tag="mm")
                    nc.tensor.matmul(rps_e[:], ones_sb[64:65, 0:64], dstage[64:65, 0, :],
                                     start=True, stop=True)
                    nc.tensor.matmul(rps_o[:], ones_sb[64:65, 0:64], dstage[64:65, 1, :],
                                     start=True, stop=True)
                    nc.scalar.activation(rps_e[:], rps_e[:], AF.Ln)
                    nc.scalar.activation(rps_o[:], rps_o[:], AF.Ln)
                    rbc_e = norm.tile([64, T], BF16, tag="rbc")
                    rbc_o = norm.tile([64, T], BF16, tag="rbc")
                    nc.scalar.activation(rbc_e[:], rps_e[:], AF.Exp, scale=-1.0)
                    nc.scalar.activation(rbc_o[:], rps_o[:], AF.Exp, scale=-1.0)
                    nc.vector.tensor_tensor(yT_c[0:64, cpair, :], yps_e[0:64, :],
                                            rbc_e[:], ALU.mult)
                    ystg = stage.tile([64, T], BF16, tag="ystg")
                    nc.vector.tensor_tensor(ystg[:], yps_o[0:64, :], rbc_o[:], ALU.mult)
                    nc.sync.dma_start(yT_c[64:128, cpair, :], ystg[:])

                # ------------- proj + residual (+ fused ln2 stats) -------------
                if has_bias['proj']:
                    bproj_sb = small.tile([128, E], F32, tag="bproj")
                    nc.sync.dma_start(bproj_sb[:], bproj_p[l])
                stats = []
                for t in range(QCH):
                    for nn in range(2):
                        NW = E // 2
                        ps = psA.tile([128, NW], F32, tag="mm")
                        for kc in range(ECH):
                            nc.tensor.matmul(ps[:], yT_c[:, kc, 128 * t:128 * (t + 1)],
                                             wp[:, kc, nn * NW:(nn + 1) * NW],
                                             start=(kc == 0), stop=(kc == ECH - 1))
                        hs = h_sb[:, t, nn * NW:(nn + 1) * NW]
                        nc.vector.tensor_tensor(hs, hs, ps[:], ALU.add)
                        if has_bias['proj']:
                            nc.vector.tensor_tensor(hs, hs, bproj_sb[:, nn * NW:(nn + 1) * NW], ALU.add)
                    stats.append(ln_stats(h_sb[:, t, :]))

                # ---------------- ln2 apply -> mT ----------------
                mT = ln_apply(stats, "xT")

                # ---------------- fc1 + gelu ----------------
                if has_bias['fc']:
                    bfc_sb = small.tile([128, FCH], F32, tag="bfc")
                    nc.sync.dma_start(bfc_sb[:], bfc_p[l].rearrange("c p one -> p (c one)"))
                gT = acts.tile([128, FCH, T], BF16, tag="gT")
                # prefetch fc2 weights during fc1
                wf2 = wpool.tile([128, FCH, E], BF16, tag="W")
                nc.scalar.dma_start(wf2[:], wfc2_p[l])
                for fm in range(FCH):
                    ps = psA.tile([128, T], F32, tag="mm")
                    for kc in range(ECH):
                        nc.tensor.matmul(ps[:], wf[:, kc, 128 * fm:128 * (fm + 1)],
                                         mT[:, kc, :],
                                         start=(kc == 0), stop=(kc == ECH - 1))
                    bias_arg = bfc_sb[:, fm:fm + 1] if has_bias['fc'] else 0.0
                    nc.scalar.activation(gT[:, fm, :], ps[:], AF.Gelu_apprx_tanh, bias=bias_arg)

                # ------- fc2 + residual (+ fused ln1/lnf stats for next) -------
                if has_bias['fc2']:
                    bfc2_sb = small.tile([128, E], F32, tag="bfc2")
                    nc.sync.dma_start(bfc2_sb[:], bfc2_p[l])
                stats = []
                for t in range(QCH):
                    for nn in range(2):
                        NW = E // 2
                        ps = psA.tile([128, NW], F32, tag="mm")
                        for kc in range(FCH):
                            nc.tensor.matmul(ps[:], gT[:, kc, 128 * t:128 * (t + 1)],
                                             wf2[:, kc, nn * NW:(nn + 1) * NW],
                                             start=(kc == 0), stop=(kc == FCH - 1))
                        hs = h_sb[:, t, nn * NW:(nn + 1) * NW]
                        nc.vector.tensor_tensor(hs, hs, ps[:], ALU.add)
                        if has_bias['fc2']:
                            nc.vector.tensor_tensor(hs, hs, bfc2_sb[:, nn * NW:(nn + 1) * NW], ALU.add)
                    stats.append(ln_stats(h_sb[:, t, :]))

            # ---------------- final ln + lm head ----------------
            hfT = ln_apply(stats, "xT")
            if has_bias['lm']:
                blm_sb = small.tile([1, VNC * 512], BF16, tag="blm")
                nc.sync.dma_start(blm_sb[:], blm_p[:])
            for n in range(VNC):
                wl = wpool.tile([128, ECH, 512], BF16, tag="W")
                nc.scalar.dma_start(wl[:], wlm_p[:, :, 512 * n:512 * (n + 1)])
                NW = min(512, V - 512 * n)
                for t in range(QCH):
                    ps = psA.tile([128, 512], F32, tag="mm")
                    for kc in range(ECH):
                        nc.tensor.matmul(ps[:], hfT[:, kc, 128 * t:128 * (t + 1)],
                                         wl[:, kc, :],
                                         start=(kc == 0), stop=(kc == ECH - 1 and not has_bias['lm']))
                    if has_bias['lm']:
                        nc.tensor.matmul(ps[:], ones_sb[0:1, 0:128],
                                         blm_sb[0:1, 512 * n:512 * (n + 1)],
                                         start=False, stop=True)
                    lstg = stage.tile([128, 512], BF16, tag="lmstg")
                    nc.vector.tensor_copy(out=lstg[:, 0:NW], in_=ps[:, 0:NW])
                    nc.sync.dma_start(
                        out_p[128 * t:128 * (t + 1), 512 * n:512 * n + NW],
                        lstg[:, 0:NW])
    return nc


# ---------------------------------------------------------------------------
# host prep
# ---------------------------------------------------------------------------

def host_prep(inputs, c):
    d = derived(c)
    B, S, L, H, D, F, V, E, T = c['B'], c['S'], c['L'], c['H'], c['D'], c['F'], c['V'], d['E'], d['T']
    ECH, FCH, QCH, KCH, VNC = d['ECH'], d['FCH'], d['QCH'], d['KCH'], d['VNC']

    f32 = lambda a: np.asarray(a, np.float32)
    x = np.asarray(inputs['x']).astype(np.int64)
    wte, wpe = f32(inputs['wte']), f32(inputs['wpe'])
    g1, b1 = f32(inputs['ln1_g']), f32(inputs['ln1_b'])
    aw, ab = f32(inputs['attn_w']), f32(inputs['attn_b'])
    pw, pb = f32(inputs['attn_proj_w']), f32(inputs['attn_proj_b'])
    g2, b2 = f32(inputs['ln2_g']), f32(inputs['ln2_b'])
    fw, fb = f32(inputs['fc_w']), f32(inputs['fc_b'])
    p2w, p2b = f32(inputs['fc_proj_w']), f32(inputs['fc_proj_b'])
    gf, bf_ = f32(inputs['lnf_g']), f32(inputs['lnf_b'])
    lm = f32(inputs['lm_head_w'])

    scale = 1.0 / np.sqrt(D)
    # fold ln1 gamma/beta into attn_w/attn_b ; scale q by 1/sqrt(D)
    aw_f = aw * g1[:, :, None]              # [L, E, 3E]
    ab_f = ab + np.einsum('le,lef->lf', b1, aw)
    aw_f[:, :, :E] *= scale
    ab_f[:, :E] *= scale
    fw_f = fw * g2[:, :, None]
    fb_f = fb + np.einsum('le,lef->lf', b2, fw)
    lm_f = lm * gf[:, None]
    blm_f = bf_ @ lm                         # [V]

    def bfc16(a):
        return np.ascontiguousarray(a).astype(BF)

    wqkv = bfc16(aw_f.reshape(L, ECH, 128, 3 * E).transpose(0, 2, 1, 3))
    wproj = bfc16(pw.reshape(L, ECH, 128, E).transpose(0, 2, 1, 3))
    wfc = bfc16(fw_f.reshape(L, ECH, 128, F).transpose(0, 2, 1, 3))
    wfc2 = bfc16(p2w.reshape(L, FCH, 128, E).transpose(0, 2, 1, 3))
    wlm_pad = np.zeros((E, VNC * 512), np.float32)
    wlm_pad[:, :V] = lm_f
    wlm = bfc16(wlm_pad.reshape(ECH, 128, VNC * 512).transpose(1, 0, 2))

    has_bias = dict(
        qkv=bool(np.any(ab_f[:, :2 * E])), v=bool(np.any(ab_f[:, 2 * E:])),
        proj=bool(np.any(pb)), fc=bool(np.any(fb_f)), fc2=bool(np.any(p2b)),
        lm=bool(np.any(blm_f)))

    # 0/1 keep-mask [128, 2, 64]: rows = key slot (r, s) in a chunk (global
    # key 128j + 2s + r), cols = query w in the 64-query diagonal block
    # (global query 2(64j + w) + p). masked iff 2s + r > 2w + p.
    def diag_mask01(p):
        k = np.arange(128)
        gk = 2 * (k % 64) + (k >= 64)
        w = np.arange(64)
        m = np.where(gk[:, None] > 2 * w[None, :] + p, 0.0, 1.0).astype(BF)
        return np.ascontiguousarray(np.stack([m, m], axis=1))

    # embeddings, strided
    emb = wte[x] + wpe[:S][None, :, :]       # [B, S, E] f32
    in_maps = []
    metas = []
    for core in range(8):
        b, p = core // 2, core % 2
        h0 = np.ascontiguousarray(emb[b, p::2, :]).astype(np.float32)
        m = dict(h0=h0, wqkv=wqkv, wproj=wproj, wfc=wfc, wfc2=wfc2, wlm=wlm,
                 mask01=diag_mask01(p), i128=np.eye(128, dtype=np.float32).astype(BF))
        if has_bias['qkv']:
            m['bqk'] = np.ascontiguousarray(
                ab_f[:, :2 * E].reshape(L, 2 * ECH, 128, 1)).astype(np.float32)
        if has_bias['v']:
            m['bv'] = ab_f[:, 2 * E:].reshape(L, 1, E).astype(BF)
        if has_bias['proj']:
            m['bproj'] = np.tile(pb[:, None, :], (1, 128, 1)).astype(np.float32)
        if has_bias['fc']:
            m['bfc'] = fb_f.reshape(L, FCH, 128, 1).astype(np.float32)
        if has_bias['fc2']:
            m['bfc2'] = np.tile(p2b[:, None, :], (1, 128, 1)).astype(np.float32)
        if has_bias['lm']:
            blm_pad = np.zeros((1, VNC * 512), np.float32)
            blm_pad[0, :V] = blm_f
            m['blm'] = blm_pad.astype(BF)
        in_maps.append(m)
        metas.append((b, p))
    return in_maps, metas, has_bias


def run(inputs, c, nc=None, has_bias=None, in_maps=None, metas=None, trace=False):
    if in_maps is None:
        in_maps, metas, has_bias = host_prep(inputs, c)
    if nc is None:
        nc = build(c, has_bias)
        nc.compile()
    res = run_bass_kernel_spmd(nc, in_maps, core_ids=list(range(8)), trace=trace)
    B, S, V = c['B'], c['S'], c['V']
    out = np.empty((B, S, V), np.float32)
    for core in range(8):
        b, p = metas[core]
        out[b, p::2, :] = np.asarray(res.results[core]["logits"]).astype(np.float32)
    return out, nc, res


# ---------------------------------------------------------------------------
# harness entry point: kernel(**inputs) -> full logits [B, S, V] float32
# ---------------------------------------------------------------------------
_NC_CACHE = {}


def kernel(**inputs):
    c = cfg_full()
    in_maps, metas, has_bias = host_prep(inputs, c)
    key = tuple(sorted(has_bias.items()))
    if key not in _NC_CACHE:
        nc = build(c, has_bias)
        nc.compile()
        _NC_CACHE[key] = nc
    nc = _NC_CACHE[key]
    res = run_bass_kernel_spmd(nc, in_maps, core_ids=list(range(8)))
    B, S, V = c['B'], c['S'], c['V']
    out = np.empty((B, S, V), np.float32)
    for core in range(8):
        b, p = metas[core]
        out[b, p::2, :] = np.asarray(res.results[core]["logits"]).astype(np.float32)
    return out
